# revision 2
# baseline (speedup 1.0000x reference)
"""Multi-head attention (B=2, S=2048, H=16, D=64) on 8 TRN2 NeuronCores.

Sharding: data parallel on batch (2) x tensor parallel on heads (16 -> 4 per
core).  Core c handles batch c//4 and heads [4*(c%4), 4*(c%4)+4).  Each core
projects q/k/v for its head group from its batch's activations, runs the
full S x S attention for its 4 heads, and writes ctx in [head, D, S] layout.
The host transposes/concatenates shards (not part of HW exec time).

Device kernel (per core, identical SPMD program, no collectives):
  - qT/kT computed directly in [D, S] layout; head pair packed into 128
    partitions (head 2p on partitions 0:64, head 2p+1 on 64:128).
  - scoresT per key chunk via a row-tiled CONCURRENT matmul pair: head a on
    PE array rows 0:63 (tile_position (0,0)), head b on rows 64:127
    ((64,0)) -> both [128 keys, 512 queries] matmuls stream together at
    ~1x cost (measured 108ns/MM vs 216 serial).
  - the two outputs land in one [128, 2, 512] PSUM tile (2 banks); ONE
    scalar-engine Exp (N=1024, scale=1/sqrt(D)) converts both heads' scores
    to bf16 probs.  The ACT engine is the kernel bottleneck (~1.1us/patch,
    128 patches); everything else is scheduled to hide under it.
  - softmax denominator via 64 ones-columns appended to v (the ctx matmul
    emits the denominator replicated on psum partitions 64:128 at no cost;
    matmul cost is N-bound).
  - padding mask folded into v_aug row zeroing (exp(x-1e4) underflows to 0
    in f32, so zeroing masked key rows is exactly equivalent).
  - DMA issued in priority order on the HWDGE FIFO (pair-0 wk/wq halves,
    x column-chunk 0, wv, remaining halves/chunks) so the first projections
    start ~4us in; q/k/v projections are emitted just-in-time between
    attention patches at half-chunk granularity to avoid ACT stalls.
"""

import numpy as np
import ml_dtypes

import concourse.bass as bass
import concourse.tile as tile
from concourse import bacc, mybir
from concourse.bass_utils import run_bass_kernel_spmd

B, S, H, D = 2, 2048, 16, 64
HID = H * D
NCORES = 8
HPC = 4               # heads per core
COLS = HPC * D        # 256 projection columns per core
KC = HID // 128       # 8 contraction chunks for projections
QC = S // 512         # 4 query chunks of 512
MC = S // 128         # 16 key chunks of 128

BF16 = mybir.dt.bfloat16
F32 = mybir.dt.float32
np_bf16 = ml_dtypes.bfloat16

_CACHE = {}


def build(apply_mask: bool) -> bass.Bass:
    nc = bacc.Bacc(None, target_bir_lowering=False, debug=False)

    xT = nc.declare_dram_parameter("xT", [HID, S], BF16, isOutput=False)
    wq = nc.declare_dram_parameter("wq", [HID, COLS], BF16, isOutput=False)
    wk = nc.declare_dram_parameter("wk", [HID, COLS], BF16, isOutput=False)
    wv = nc.declare_dram_parameter("wv", [HID, COLS], BF16, isOutput=False)
    bq = nc.declare_dram_parameter("bq", [128, 2], F32, isOutput=False)
    bk = nc.declare_dram_parameter("bk", [128, 2], F32, isOutput=False)
    bv = nc.declare_dram_parameter("bv", [128, COLS], F32, isOutput=False)
    if apply_mask:
        mm_in = nc.declare_dram_parameter("maskm", [128, MC], F32, isOutput=False)
    out_ext = nc.declare_dram_parameter("out", [HPC, D, S], F32, isOutput=True)

    with tile.TileContext(nc) as tc:
        with (
            tc.tile_pool(name="singles", bufs=1) as singles,
            tc.tile_pool(name="work", bufs=4) as work,
            tc.tile_pool(name="psum", bufs=2, space="PSUM") as psum,
        ):
            # ---- input DMA, strict priority order (HWDGE is FIFO) ----
            bq_sb = singles.tile([128, 2], F32)
            nc.sync.dma_start(out=bq_sb, in_=bq[:, :])
            bk_sb = singles.tile([128, 2], F32)
            nc.sync.dma_start(out=bk_sb, in_=bk[:, :])
            bv_sb = singles.tile([128, COLS], F32)
            nc.sync.dma_start(out=bv_sb, in_=bv[:, :])
            if apply_mask:
                mm_sb = singles.tile([128, MC], F32)
                nc.sync.dma_start(out=mm_sb, in_=mm_in[:, :])

            wq_sb = singles.tile([128, KC, COLS], BF16)
            wk_sb = singles.tile([128, KC, COLS], BF16)
            wv_sb = singles.tile([128, KC, COLS], BF16)
            x_sb = singles.tile([128, KC, S], BF16)

            def dma_w_half(w_sb, w_ext, h):
                csl = slice(h * 128, (h + 1) * 128)
                for kc in range(KC):
                    nc.sync.dma_start(out=w_sb[:, kc, csl],
                                      in_=w_ext[kc * 128:(kc + 1) * 128, csl])

            def dma_x(cq):
                csl = slice(cq * 512, (cq + 1) * 512)
                for kc in range(KC):
                    nc.sync.dma_start(out=x_sb[:, kc, csl],
                                      in_=xT[kc * 128:(kc + 1) * 128, csl])

            dma_w_half(wk_sb, wk, 0)
            dma_w_half(wq_sb, wq, 0)
            dma_x(0)
            for kc in range(KC):
                nc.sync.dma_start(out=wv_sb[:, kc, :], in_=wv[kc * 128:(kc + 1) * 128, :])
            dma_x(1)
            dma_w_half(wk_sb, wk, 1)
            dma_w_half(wq_sb, wq, 1)
            dma_x(2)
            dma_x(3)

            # HAM warm-up: burn ~4us of dummy matmuls on the first-arriving
            # weight half so the PE is at full clock when real work starts;
            # output is never read.
            warm_ps = psum.tile([128, 128], F32, tag="proj", name="warm_ps")
            for i in range(40):
                nc.tensor.matmul(warm_ps, lhsT=wk_sb[:, 0, 0:128],
                                 rhs=wk_sb[:, 0, 0:128],
                                 start=(i == 0), stop=(i == 39))

            # v_aug: [128, key_chunk, head, 128]; cols 64:128 are ones columns,
            # so the ctx matmul emits the softmax denominator replicated into
            # psum partitions 64:128 at no extra cost (matmul cost is N-bound)
            v_aug = singles.tile([128, MC, HPC, 128], BF16)
            nc.vector.memset(v_aug[:, :, :, 64:128], 1.0)

            kT = singles.tile([128, 2, S], BF16)
            qT = singles.tile([128, 2, S], BF16)

            def project_kqT(dst, w_sb, b_sb, p, c):
                """kT/qT projection for head pair p, 512-position chunk c.
                Returns two half-tasks (4 matmuls each) for fine-grained
                interleaving between attention patches."""
                csl = slice(c * 512, (c + 1) * 512)
                ps_box = []

                def half0():
                    ps = psum.tile([128, 512], F32, tag="proj",
                                   name=f"pt{nc.next_id()}")
                    ps_box.append(ps)
                    for kc in range(4):
                        nc.tensor.matmul(
                            ps, lhsT=w_sb[:, kc, p * 128:(p + 1) * 128],
                            rhs=x_sb[:, kc, csl], start=(kc == 0), stop=False)

                def half1():
                    ps = ps_box[0]
                    for kc in range(4, KC):
                        nc.tensor.matmul(
                            ps, lhsT=w_sb[:, kc, p * 128:(p + 1) * 128],
                            rhs=x_sb[:, kc, csl], start=False, stop=(kc == KC - 1))
                    nc.vector.tensor_tensor(
                        out=dst[:, p, csl], in0=ps,
                        in1=b_sb[:, p:p + 1].to_broadcast([128, 512]),
                        op=mybir.AluOpType.add)

                return half0, half1

            def project_v(mc):
                ps = psum.tile([128, COLS], F32, tag="proj",
                               name=f"pv{nc.next_id()}")
                for kc in range(KC):
                    nc.tensor.matmul(
                        ps, lhsT=x_sb[:, kc, mc * 128:(mc + 1) * 128],
                        rhs=wv_sb[:, kc, :], start=(kc == 0), stop=(kc == KC - 1))
                nc.vector.tensor_tensor(
                    out=v_aug[:, mc, :, 0:64],
                    in0=ps[:, :].rearrange("p (h d) -> p h d", h=HPC),
                    in1=bv_sb.rearrange("p (h d) -> p h d", h=HPC),
                    op=mybir.AluOpType.add)
                if apply_mask:
                    nc.vector.tensor_tensor(
                        out=v_aug[:, mc, :, :],
                        in0=v_aug[:, mc, :, :],
                        in1=mm_sb[:, mc:mc + 1, None].to_broadcast([128, HPC, 128]),
                        op=mybir.AluOpType.mult)

            # ---- just-in-time projection schedule ----
            # sched[(p, qc, kc)] = list of tasks emitted before that patch
            sched = {}

            def add_task(p, qc, kc, fn):
                sched.setdefault((p, qc, kc), []).append(fn)

            def add_kqT(slot_list, dst, w_sb, b_sb, p, c):
                h0, h1 = project_kqT(dst, w_sb, b_sb, p, c)
                add_task(*slot_list[0], h0)
                add_task(*slot_list[1], h1)

            # pair-0 qc0: v chunks 4..15 JIT (chunk mc emitted 2 patches
            # ahead of its ctx use), kT chunks 1-3 ahead of their first
            # score use (patch kc=4c), qT qc1 late in qc0.
            for mc2 in range(4, MC):
                add_task(0, 0, mc2 - 2, lambda mc2=mc2: project_v(mc2))
            add_kqT([(0, 0, 1), (0, 0, 2)], kT, wk_sb, bk_sb, 0, 1)
            add_kqT([(0, 0, 5), (0, 0, 6)], kT, wk_sb, bk_sb, 0, 2)
            add_kqT([(0, 0, 9), (0, 0, 10)], kT, wk_sb, bk_sb, 0, 3)
            add_kqT([(0, 0, 14), (0, 0, 15)], qT, wq_sb, bq_sb, 0, 1)
            # pair-0 qc1: kT pair 1 chunks 0-1, qT qc2
            add_kqT([(0, 1, 1), (0, 1, 3)], kT, wk_sb, bk_sb, 1, 0)
            add_kqT([(0, 1, 5), (0, 1, 7)], kT, wk_sb, bk_sb, 1, 1)
            add_kqT([(0, 1, 9), (0, 1, 11)], qT, wq_sb, bq_sb, 0, 2)
            # pair-0 qc2: kT pair 1 chunks 2-3, qT qc3
            add_kqT([(0, 2, 1), (0, 2, 3)], kT, wk_sb, bk_sb, 1, 2)
            add_kqT([(0, 2, 5), (0, 2, 7)], kT, wk_sb, bk_sb, 1, 3)
            add_kqT([(0, 2, 9), (0, 2, 11)], qT, wq_sb, bq_sb, 0, 3)
            # pair-0 qc3: qT pair 1 qc0, qc1
            add_kqT([(0, 3, 1), (0, 3, 3)], qT, wq_sb, bq_sb, 1, 0)
            add_kqT([(0, 3, 5), (0, 3, 7)], qT, wq_sb, bq_sb, 1, 1)
            # pair-1 qc0/qc1: qT pair 1 qc2, qc3
            add_kqT([(1, 0, 1), (1, 0, 3)], qT, wq_sb, bq_sb, 1, 2)
            add_kqT([(1, 1, 1), (1, 1, 3)], qT, wq_sb, bq_sb, 1, 3)

            def attention(p):
                ha, hb = 2 * p, 2 * p + 1
                for qc in range(QC):
                    qsl = slice(qc * 512, (qc + 1) * 512)
                    ctx_a = psum.tile([128, 512], F32, tag="ctx",
                                      name=f"ca{nc.next_id()}")
                    ctx_b = psum.tile([128, 512], F32, tag="ctx",
                                      name=f"cb{nc.next_id()}")
                    for kc in range(MC):
                        for task in sched.pop((p, qc, kc), ()):
                            task()
                        ksl = slice(kc * 128, (kc + 1) * 128)
                        s = psum.tile([128, 2, 512], F32, tag="sps",
                                      name=f"s{nc.next_id()}")
                        # row-tiled concurrent pair: head a on PE rows 0:63,
                        # head b on rows 64:127
                        nc.tensor.matmul(s[:, 0, :], lhsT=kT[0:64, p, ksl],
                                         rhs=qT[0:64, p, qsl],
                                         start=True, stop=True)
                        nc.tensor.matmul(s[:, 1, :], lhsT=kT[64:128, p, ksl],
                                         rhs=qT[64:128, p, qsl],
                                         start=True, stop=True)
                        e = work.tile([128, 2, 512], BF16, tag="expT",
                                      name=f"e{nc.next_id()}")
                        nc.scalar.activation(e, s, mybir.ActivationFunctionType.Exp,
                                             scale=0.125)
                        nc.tensor.matmul(ctx_a, lhsT=v_aug[:, kc, ha, :],
                                         rhs=e[:, 0, :],
                                         start=(kc == 0), stop=(kc == MC - 1))
                        nc.tensor.matmul(ctx_b, lhsT=v_aug[:, kc, hb, :],
                                         rhs=e[:, 1, :],
                                         start=(kc == 0), stop=(kc == MC - 1))
                    for h, ctx in ((ha, ctx_a), (hb, ctx_b)):
                        # one copy releases the ctx psum bank immediately;
                        # custom-DVE recip needs a base-0 input tile
                        g_sb = work.tile([128, 512], F32, tag="gctx",
                                         name=f"g{nc.next_id()}")
                        nc.vector.tensor_copy(out=g_sb, in_=ctx)
                        d0_sb = work.tile([64, 512], F32, tag="den0",
                                          name=f"d0{nc.next_id()}")
                        nc.vector.tensor_copy(out=d0_sb, in_=g_sb[64:128, :])
                        d_sb = work.tile([64, 512], F32, tag="den",
                                         name=f"d{nc.next_id()}")
                        nc.vector.reciprocal_approx_fast(out=d_sb, in_=d0_sb)
                        o_sb = work.tile([64, 512], F32, tag="outt",
                                         name=f"o{nc.next_id()}")
                        nc.vector.tensor_tensor(out=o_sb, in0=g_sb[0:64, :],
                                                in1=d_sb, op=mybir.AluOpType.mult)
                        nc.sync.dma_start(out=out_ext[h][:, qsl], in_=o_sb)

            # initial projections: pair-0 kT/qT chunk 0 + v chunks 0-3
            h0, h1 = project_kqT(kT, wk_sb, bk_sb, 0, 0)
            h0(); h1()
            h0, h1 = project_kqT(qT, wq_sb, bq_sb, 0, 0)
            h0(); h1()
            for mc in range(4):
                project_v(mc)

            attention(0)
            attention(1)
            assert not sched, f"unconsumed proj tasks: {list(sched)}"

    nc.compile()
    return nc


def _get_nc(apply_mask: bool) -> bass.Bass:
    if apply_mask not in _CACHE:
        _CACHE[apply_mask] = build(apply_mask)
    return _CACHE[apply_mask]


def _in_maps(x, mask, Wq, bq, Wk, bk, Wv, bv, apply_mask):
    xT_b = [np.ascontiguousarray(x[b].T).astype(np_bf16) for b in range(B)]
    maps = []
    for c in range(NCORES):
        b, hg = c // 4, c % 4
        cs = slice(hg * COLS, (hg + 1) * COLS)
        m = {
            "xT": xT_b[b],
            "wq": np.ascontiguousarray(Wq[:, cs]).astype(np_bf16),
            "wk": np.ascontiguousarray(Wk[:, cs]).astype(np_bf16),
            "wv": np.ascontiguousarray(Wv[:, cs]).astype(np_bf16),
            "bq": np.ascontiguousarray(bq[cs].reshape(2, 128).T).astype(np.float32),
            "bk": np.ascontiguousarray(bk[cs].reshape(2, 128).T).astype(np.float32),
            "bv": np.ascontiguousarray(
                np.broadcast_to(bv[cs], (128, COLS))).astype(np.float32),
        }
        if apply_mask:
            m["maskm"] = np.ascontiguousarray(
                mask[b].astype(np.float32).reshape(MC, 128).T)
        maps.append(m)
    return maps


def _ensure_ntff_hook():
    """The agent image's antenv lacks axon_hooks; synthesize it so
    run_bass_kernel_spmd(trace=True) can reach the axon NTFF profiler."""
    import sys as _sys
    import types as _types
    try:
        from antenv import axon_hooks  # noqa: F401
        return
    except ImportError:
        pass
    import antenv
    mod = _types.ModuleType("antenv.axon_hooks")
    _hook = [None]
    mod.set_axon_ntff_profile_hook = lambda h: _hook.__setitem__(0, h)
    mod.get_axon_ntff_profile_hook = lambda: _hook[0]
    _sys.modules["antenv.axon_hooks"] = mod
    antenv.axon_hooks = mod
    from trn_agent_boot.trn_boot import _ntff_profile_via_ctypes
    mod.set_axon_ntff_profile_hook(
        _ntff_profile_via_ctypes("/opt/axon/libaxon_pjrt.so"))


def run(inputs: dict, trace: bool = False):
    if trace:
        _ensure_ntff_hook()
    x = np.asarray(inputs["x"], dtype=np.float32)
    mask = np.asarray(inputs["mask"])
    apply_mask = not bool((mask == 1).all())
    nc = _get_nc(apply_mask)
    maps = _in_maps(x, mask, np.asarray(inputs["Wq"], np.float32),
                    np.asarray(inputs["bq"], np.float32),
                    np.asarray(inputs["Wk"], np.float32),
                    np.asarray(inputs["bk"], np.float32),
                    np.asarray(inputs["Wv"], np.float32),
                    np.asarray(inputs["bv"], np.float32), apply_mask)
    res = run_bass_kernel_spmd(nc, maps, core_ids=list(range(NCORES)), trace=trace)
    out = np.empty((B, S, HID), dtype=np.float32)
    for c in range(NCORES):
        b, hg = c // 4, c % 4
        cs = slice(hg * COLS, (hg + 1) * COLS)
        ctxT = res.results[c]["out"]          # [HPC, D, S]
        out[b, :, cs] = ctxT.transpose(2, 0, 1).reshape(S, COLS)
    return out, res


def kernel(**inputs) -> np.ndarray:
    out, _ = run(inputs)
    return out


# revision 3
# speedup vs baseline: 1.0676x; 1.0676x over previous
"""Multi-head attention (B=2, S=2048, H=16, D=64) on 8 TRN2 NeuronCores.

Sharding: data parallel on batch (2) x tensor parallel on heads (16 -> 4 per
core).  Core c handles batch c//4 and heads [4*(c%4), 4*(c%4)+4).  Each core
projects q/k/v for its head group from its batch's activations, runs the
full S x S attention for its 4 heads, and writes ctx in [head, D, S] layout.
The host transposes/concatenates shards (not part of HW exec time).

Device kernel (per core, identical SPMD program, no collectives):
  - qT/kT computed directly in [D, S] layout; head pair packed into 128
    partitions (head 2p on partitions 0:64, head 2p+1 on 64:128).
  - scoresT per key chunk via a row-tiled CONCURRENT matmul pair: head a on
    PE array rows 0:63 (tile_position (0,0)), head b on rows 64:127
    ((64,0)) -> both [128 keys, 512 queries] matmuls stream together at
    ~1x cost (measured 108ns/MM vs 216 serial).
  - the two outputs land in one [128, 2, 512] PSUM tile (2 banks); ONE
    scalar-engine Exp (N=1024, scale=1/sqrt(D)) converts both heads' scores
    to bf16 probs.  The ACT engine is the kernel bottleneck (~1.1us/patch,
    128 patches); everything else is scheduled to hide under it.
  - softmax denominator via 64 ones-columns appended to v (the ctx matmul
    emits the denominator replicated on psum partitions 64:128 at no cost;
    matmul cost is N-bound).
  - padding mask folded into v_aug row zeroing (exp(x-1e4) underflows to 0
    in f32, so zeroing masked key rows is exactly equivalent).
  - DMA issued in priority order on the HWDGE FIFO (pair-0 wk/wq halves,
    x column-chunk 0, wv, remaining halves/chunks) so the first projections
    start ~4us in; q/k/v projections are emitted just-in-time between
    attention patches at half-chunk granularity to avoid ACT stalls.
"""

import numpy as np
import ml_dtypes

import concourse.bass as bass
import concourse.tile as tile
from concourse import bacc, mybir
from concourse.bass_utils import run_bass_kernel_spmd

B, S, H, D = 2, 2048, 16, 64
HID = H * D
NCORES = 8
HPC = 4               # heads per core
COLS = HPC * D        # 256 projection columns per core
KC = HID // 128       # 8 contraction chunks for projections
QC = S // 512         # 4 query chunks of 512
MC = S // 128         # 16 key chunks of 128

BF16 = mybir.dt.bfloat16
F32 = mybir.dt.float32
np_bf16 = ml_dtypes.bfloat16

_CACHE = {}


def build(apply_mask: bool) -> bass.Bass:
    nc = bacc.Bacc(None, target_bir_lowering=False, debug=False)

    xT = nc.declare_dram_parameter("xT", [HID, S], BF16, isOutput=False)
    wq = nc.declare_dram_parameter("wq", [HID, COLS], BF16, isOutput=False)
    wk = nc.declare_dram_parameter("wk", [HID, COLS], BF16, isOutput=False)
    wv = nc.declare_dram_parameter("wv", [HID, COLS], BF16, isOutput=False)
    bq = nc.declare_dram_parameter("bq", [128, 2], F32, isOutput=False)
    bk = nc.declare_dram_parameter("bk", [128, 2], F32, isOutput=False)
    bv = nc.declare_dram_parameter("bv", [128, COLS], F32, isOutput=False)
    if apply_mask:
        mm_in = nc.declare_dram_parameter("maskm", [128, MC], F32, isOutput=False)
    out_ext = nc.declare_dram_parameter("out", [HPC, D, S], F32, isOutput=True)

    with tile.TileContext(nc) as tc:
        with (
            tc.tile_pool(name="singles", bufs=1) as singles,
            tc.tile_pool(name="work", bufs=4) as work,
            tc.tile_pool(name="psum", bufs=2, space="PSUM") as psum,
        ):
            # ---- input DMA, strict priority order, one dma_start per tensor
            # chunk (each dispatch costs ~600ns of serial sequencer time, so
            # fewer+bigger wins; a single dma_start fans over all 16 SDMA
            # engines anyway) ----
            bv_sb = singles.tile([128, COLS], F32)
            nc.sync.dma_start(out=bv_sb, in_=bv[:, :])
            bq_sb = singles.tile([128, 2], F32)
            nc.sync.dma_start(out=bq_sb, in_=bq[:, :])
            bk_sb = singles.tile([128, 2], F32)
            nc.sync.dma_start(out=bk_sb, in_=bk[:, :])
            if apply_mask:
                mm_sb = singles.tile([128, MC], F32)
                nc.sync.dma_start(out=mm_sb, in_=mm_in[:, :])

            wq_sb = singles.tile([128, KC, COLS], BF16)
            wk_sb = singles.tile([128, KC, COLS], BF16)
            wv_sb = singles.tile([128, KC, COLS], BF16)
            x_sb = singles.tile([128, KC, S], BF16)

            def dma_w(w_sb, w_ext):
                nc.sync.dma_start(
                    out=w_sb,
                    in_=w_ext.rearrange("(kc p) c -> p kc c", p=128))

            def dma_x(cq):
                csl = slice(cq * 512, (cq + 1) * 512)
                nc.sync.dma_start(
                    out=x_sb[:, :, csl],
                    in_=xT.rearrange("(kc p) c -> p kc c", p=128)[:, :, csl])

            dma_w(wk_sb, wk)
            dma_x(0)
            dma_w(wq_sb, wq)
            dma_w(wv_sb, wv)
            dma_x(1)
            dma_x(2)
            dma_x(3)

            # HAM warm-up: burn ~4us of dummy matmuls on the first-arriving
            # tensor (bv, tiny) so the PE is at full clock when real work
            # starts; output is never read.
            warm_ps = psum.tile([128, 128], F32, tag="proj", name="warm_ps")
            for i in range(40):
                nc.tensor.matmul(warm_ps, lhsT=bv_sb[:, 0:128],
                                 rhs=bv_sb[:, 0:128],
                                 start=(i == 0), stop=(i == 39))

            # v_aug: [128, key_chunk, head, 128]; cols 64:128 are ones columns,
            # so the ctx matmul emits the softmax denominator replicated into
            # psum partitions 64:128 at no extra cost (matmul cost is N-bound)
            v_aug = singles.tile([128, MC, HPC, 128], BF16)
            nc.vector.memset(v_aug[:, :, :, 64:128], 1.0)

            kT = singles.tile([128, 2, S], BF16)
            qT = singles.tile([128, 2, S], BF16)

            def project_kqT(dst, w_sb, b_sb, p, c):
                """kT/qT projection for head pair p, 512-position chunk c.
                Returns two half-tasks (4 matmuls each) for fine-grained
                interleaving between attention patches."""
                csl = slice(c * 512, (c + 1) * 512)
                ps_box = []

                def half0():
                    ps = psum.tile([128, 512], F32, tag="proj",
                                   name=f"pt{nc.next_id()}")
                    ps_box.append(ps)
                    for kc in range(4):
                        nc.tensor.matmul(
                            ps, lhsT=w_sb[:, kc, p * 128:(p + 1) * 128],
                            rhs=x_sb[:, kc, csl], start=(kc == 0), stop=False)

                def half1():
                    ps = ps_box[0]
                    for kc in range(4, KC):
                        nc.tensor.matmul(
                            ps, lhsT=w_sb[:, kc, p * 128:(p + 1) * 128],
                            rhs=x_sb[:, kc, csl], start=False, stop=(kc == KC - 1))
                    nc.vector.tensor_tensor(
                        out=dst[:, p, csl], in0=ps,
                        in1=b_sb[:, p:p + 1].to_broadcast([128, 512]),
                        op=mybir.AluOpType.add)

                return half0, half1

            def project_v(mc):
                ps = psum.tile([128, COLS], F32, tag="proj",
                               name=f"pv{nc.next_id()}")
                for kc in range(KC):
                    nc.tensor.matmul(
                        ps, lhsT=x_sb[:, kc, mc * 128:(mc + 1) * 128],
                        rhs=wv_sb[:, kc, :], start=(kc == 0), stop=(kc == KC - 1))
                nc.vector.tensor_tensor(
                    out=v_aug[:, mc, :, 0:64],
                    in0=ps[:, :].rearrange("p (h d) -> p h d", h=HPC),
                    in1=bv_sb.rearrange("p (h d) -> p h d", h=HPC),
                    op=mybir.AluOpType.add)
                if apply_mask:
                    nc.vector.tensor_tensor(
                        out=v_aug[:, mc, :, :],
                        in0=v_aug[:, mc, :, :],
                        in1=mm_sb[:, mc:mc + 1, None].to_broadcast([128, HPC, 128]),
                        op=mybir.AluOpType.mult)

            # ---- just-in-time projection schedule ----
            # sched[(p, qc, kc)] = list of tasks emitted before that patch
            sched = {}

            def add_task(p, qc, kc, fn):
                sched.setdefault((p, qc, kc), []).append(fn)

            def add_kqT(slot_list, dst, w_sb, b_sb, p, c):
                h0, h1 = project_kqT(dst, w_sb, b_sb, p, c)
                add_task(*slot_list[0], h0)
                add_task(*slot_list[1], h1)

            # pair-0 qc0: v chunks 4..15 JIT (chunk mc emitted 2 patches
            # ahead of its ctx use), kT chunks 1-3 ahead of their first
            # score use (patch kc=4c), qT qc1 late in qc0.
            for mc2 in range(4, MC):
                add_task(0, 0, mc2 - 2, lambda mc2=mc2: project_v(mc2))
            add_kqT([(0, 0, 1), (0, 0, 2)], kT, wk_sb, bk_sb, 0, 1)
            add_kqT([(0, 0, 5), (0, 0, 6)], kT, wk_sb, bk_sb, 0, 2)
            add_kqT([(0, 0, 9), (0, 0, 10)], kT, wk_sb, bk_sb, 0, 3)
            add_kqT([(0, 0, 14), (0, 0, 15)], qT, wq_sb, bq_sb, 0, 1)
            # pair-0 qc1: kT pair 1 chunks 0-1, qT qc2
            add_kqT([(0, 1, 1), (0, 1, 3)], kT, wk_sb, bk_sb, 1, 0)
            add_kqT([(0, 1, 5), (0, 1, 7)], kT, wk_sb, bk_sb, 1, 1)
            add_kqT([(0, 1, 9), (0, 1, 11)], qT, wq_sb, bq_sb, 0, 2)
            # pair-0 qc2: kT pair 1 chunks 2-3, qT qc3
            add_kqT([(0, 2, 1), (0, 2, 3)], kT, wk_sb, bk_sb, 1, 2)
            add_kqT([(0, 2, 5), (0, 2, 7)], kT, wk_sb, bk_sb, 1, 3)
            add_kqT([(0, 2, 9), (0, 2, 11)], qT, wq_sb, bq_sb, 0, 3)
            # pair-0 qc3: qT pair 1 qc0, qc1
            add_kqT([(0, 3, 1), (0, 3, 3)], qT, wq_sb, bq_sb, 1, 0)
            add_kqT([(0, 3, 5), (0, 3, 7)], qT, wq_sb, bq_sb, 1, 1)
            # pair-1 qc0/qc1: qT pair 1 qc2, qc3
            add_kqT([(1, 0, 1), (1, 0, 3)], qT, wq_sb, bq_sb, 1, 2)
            add_kqT([(1, 1, 1), (1, 1, 3)], qT, wq_sb, bq_sb, 1, 3)

            def attention(p):
                ha, hb = 2 * p, 2 * p + 1
                for qc in range(QC):
                    qsl = slice(qc * 512, (qc + 1) * 512)
                    ctx_a = psum.tile([128, 512], F32, tag="ctx",
                                      name=f"ca{nc.next_id()}")
                    ctx_b = psum.tile([128, 512], F32, tag="ctx",
                                      name=f"cb{nc.next_id()}")
                    for kc in range(MC):
                        for task in sched.pop((p, qc, kc), ()):
                            task()
                        ksl = slice(kc * 128, (kc + 1) * 128)
                        s = psum.tile([128, 2, 512], F32, tag="sps",
                                      name=f"s{nc.next_id()}")
                        # row-tiled concurrent pair: head a on PE rows 0:63,
                        # head b on rows 64:127
                        nc.tensor.matmul(s[:, 0, :], lhsT=kT[0:64, p, ksl],
                                         rhs=qT[0:64, p, qsl],
                                         start=True, stop=True)
                        nc.tensor.matmul(s[:, 1, :], lhsT=kT[64:128, p, ksl],
                                         rhs=qT[64:128, p, qsl],
                                         start=True, stop=True)
                        e = work.tile([128, 2, 512], BF16, tag="expT",
                                      name=f"e{nc.next_id()}")
                        nc.scalar.activation(e, s, mybir.ActivationFunctionType.Exp,
                                             scale=0.125)
                        nc.tensor.matmul(ctx_a, lhsT=v_aug[:, kc, ha, :],
                                         rhs=e[:, 0, :],
                                         start=(kc == 0), stop=(kc == MC - 1))
                        nc.tensor.matmul(ctx_b, lhsT=v_aug[:, kc, hb, :],
                                         rhs=e[:, 1, :],
                                         start=(kc == 0), stop=(kc == MC - 1))
                    for h, ctx in ((ha, ctx_a), (hb, ctx_b)):
                        # one copy releases the ctx psum bank immediately;
                        # custom-DVE recip needs a base-0 input tile
                        g_sb = work.tile([128, 512], F32, tag="gctx",
                                         name=f"g{nc.next_id()}")
                        nc.vector.tensor_copy(out=g_sb, in_=ctx)
                        d0_sb = work.tile([64, 512], F32, tag="den0",
                                          name=f"d0{nc.next_id()}")
                        nc.vector.tensor_copy(out=d0_sb, in_=g_sb[64:128, :])
                        d_sb = work.tile([64, 512], F32, tag="den",
                                         name=f"d{nc.next_id()}")
                        nc.vector.reciprocal_approx_fast(out=d_sb, in_=d0_sb)
                        o_sb = work.tile([64, 512], F32, tag="outt",
                                         name=f"o{nc.next_id()}")
                        nc.vector.tensor_tensor(out=o_sb, in0=g_sb[0:64, :],
                                                in1=d_sb, op=mybir.AluOpType.mult)
                        nc.sync.dma_start(out=out_ext[h][:, qsl], in_=o_sb)

            # initial projections: pair-0 kT/qT chunk 0 + v chunks 0-3
            h0, h1 = project_kqT(kT, wk_sb, bk_sb, 0, 0)
            h0(); h1()
            h0, h1 = project_kqT(qT, wq_sb, bq_sb, 0, 0)
            h0(); h1()
            for mc in range(4):
                project_v(mc)

            attention(0)
            attention(1)
            assert not sched, f"unconsumed proj tasks: {list(sched)}"

    nc.compile()
    return nc


def _get_nc(apply_mask: bool) -> bass.Bass:
    if apply_mask not in _CACHE:
        _CACHE[apply_mask] = build(apply_mask)
    return _CACHE[apply_mask]


def _in_maps(x, mask, Wq, bq, Wk, bk, Wv, bv, apply_mask):
    xT_b = [np.ascontiguousarray(x[b].T).astype(np_bf16) for b in range(B)]
    maps = []
    for c in range(NCORES):
        b, hg = c // 4, c % 4
        cs = slice(hg * COLS, (hg + 1) * COLS)
        m = {
            "xT": xT_b[b],
            "wq": np.ascontiguousarray(Wq[:, cs]).astype(np_bf16),
            "wk": np.ascontiguousarray(Wk[:, cs]).astype(np_bf16),
            "wv": np.ascontiguousarray(Wv[:, cs]).astype(np_bf16),
            "bq": np.ascontiguousarray(bq[cs].reshape(2, 128).T).astype(np.float32),
            "bk": np.ascontiguousarray(bk[cs].reshape(2, 128).T).astype(np.float32),
            "bv": np.ascontiguousarray(
                np.broadcast_to(bv[cs], (128, COLS))).astype(np.float32),
        }
        if apply_mask:
            m["maskm"] = np.ascontiguousarray(
                mask[b].astype(np.float32).reshape(MC, 128).T)
        maps.append(m)
    return maps


def _ensure_ntff_hook():
    """The agent image's antenv lacks axon_hooks; synthesize it so
    run_bass_kernel_spmd(trace=True) can reach the axon NTFF profiler."""
    import sys as _sys
    import types as _types
    try:
        from antenv import axon_hooks  # noqa: F401
        return
    except ImportError:
        pass
    import antenv
    mod = _types.ModuleType("antenv.axon_hooks")
    _hook = [None]
    mod.set_axon_ntff_profile_hook = lambda h: _hook.__setitem__(0, h)
    mod.get_axon_ntff_profile_hook = lambda: _hook[0]
    _sys.modules["antenv.axon_hooks"] = mod
    antenv.axon_hooks = mod
    from trn_agent_boot.trn_boot import _ntff_profile_via_ctypes
    mod.set_axon_ntff_profile_hook(
        _ntff_profile_via_ctypes("/opt/axon/libaxon_pjrt.so"))


def run(inputs: dict, trace: bool = False):
    if trace:
        _ensure_ntff_hook()
    x = np.asarray(inputs["x"], dtype=np.float32)
    mask = np.asarray(inputs["mask"])
    apply_mask = not bool((mask == 1).all())
    nc = _get_nc(apply_mask)
    maps = _in_maps(x, mask, np.asarray(inputs["Wq"], np.float32),
                    np.asarray(inputs["bq"], np.float32),
                    np.asarray(inputs["Wk"], np.float32),
                    np.asarray(inputs["bk"], np.float32),
                    np.asarray(inputs["Wv"], np.float32),
                    np.asarray(inputs["bv"], np.float32), apply_mask)
    res = run_bass_kernel_spmd(nc, maps, core_ids=list(range(NCORES)), trace=trace)
    out = np.empty((B, S, HID), dtype=np.float32)
    for c in range(NCORES):
        b, hg = c // 4, c % 4
        cs = slice(hg * COLS, (hg + 1) * COLS)
        ctxT = res.results[c]["out"]          # [HPC, D, S]
        out[b, :, cs] = ctxT.transpose(2, 0, 1).reshape(S, COLS)
    return out, res


def kernel(**inputs) -> np.ndarray:
    out, _ = run(inputs)
    return out


# revision 9
# speedup vs baseline: 2.0209x; 1.8929x over previous
"""Multi-head attention (B=2, S=2048, H=16, D=64) on 8 TRN2 NeuronCores.

Sharding: data parallel on batch (2) x tensor parallel on heads (16 -> 4 per
core).  Core c handles batch c//4 and heads [4*(c%4), 4*(c%4)+4).  Each core
projects q/k/v for its head group from its batch's activations, runs the
full S x S attention for its 4 heads, and writes ctx in [head, D, S] layout.
The host transposes/concatenates shards (not part of HW exec time).

Device kernel (per core, identical SPMD program, no collectives):
  - qT/kT computed directly in [D, S] layout; head pair packed into 128
    partitions (head 2p on partitions 0:64, head 2p+1 on 64:128).
  - scoresT per key chunk via a row-tiled CONCURRENT matmul pair: head a on
    PE array rows 0:63 (tile_position (0,0)), head b on rows 64:127
    ((64,0)) -> both [128 keys, 512 queries] matmuls stream together at
    ~1x cost (measured 108ns/MM vs 216 serial).
  - the two outputs land in one [128, 2, 512] PSUM tile (2 banks); ONE
    scalar-engine Exp (N=1024, scale=1/sqrt(D)) converts both heads' scores
    to bf16 probs.  The ACT engine is the kernel bottleneck (~1.1us/patch,
    128 patches); everything else is scheduled to hide under it.
  - softmax denominator via 64 ones-columns appended to v (the ctx matmul
    emits the denominator replicated on psum partitions 64:128 at no cost;
    matmul cost is N-bound).
  - padding mask folded into v_aug row zeroing (exp(x-1e4) underflows to 0
    in f32, so zeroing masked key rows is exactly equivalent).
  - DMA issued in priority order on the HWDGE FIFO (pair-0 wk/wq halves,
    x column-chunk 0, wv, remaining halves/chunks) so the first projections
    start ~4us in; q/k/v projections are emitted just-in-time between
    attention patches at half-chunk granularity to avoid ACT stalls.
"""

import numpy as np
import ml_dtypes

import concourse.bass as bass
import concourse.tile as tile
from concourse import bacc, mybir
from concourse.bass_utils import run_bass_kernel_spmd

B, S, H, D = 2, 2048, 16, 64
HID = H * D
NCORES = 8
HPC = 4               # heads per core
COLS = HPC * D        # 256 projection columns per core
KC = HID // 128       # 8 contraction chunks for projections
QC = S // 512         # 4 query chunks of 512
MC = S // 128         # 16 key chunks of 128

BF16 = mybir.dt.bfloat16
F32 = mybir.dt.float32
np_bf16 = ml_dtypes.bfloat16

_CACHE = {}


def build(apply_mask: bool) -> bass.Bass:
    nc = bacc.Bacc(None, target_bir_lowering=False, debug=False)

    xT = nc.declare_dram_parameter("xT", [HID, S], BF16, isOutput=False)
    wq = nc.declare_dram_parameter("wq", [HID, COLS], BF16, isOutput=False)
    wk = nc.declare_dram_parameter("wk", [HID, COLS], BF16, isOutput=False)
    wv = nc.declare_dram_parameter("wv", [HID, COLS], BF16, isOutput=False)
    bq = nc.declare_dram_parameter("bq", [128, 2], F32, isOutput=False)
    bk = nc.declare_dram_parameter("bk", [128, 2], F32, isOutput=False)
    bv = nc.declare_dram_parameter("bv", [128, COLS], F32, isOutput=False)
    if apply_mask:
        mm_in = nc.declare_dram_parameter("maskm", [128, MC], F32, isOutput=False)
    out_ext = nc.declare_dram_parameter("out", [HPC, D, S], F32, isOutput=True)

    with tile.TileContext(nc) as tc:
        with (
            tc.tile_pool(name="singles", bufs=1) as singles,
            tc.tile_pool(name="work", bufs=4) as work,
            tc.tile_pool(name="psum", bufs=2, space="PSUM") as psum,
        ):
            # ---- input DMA, strict priority order, one dma_start per tensor
            # chunk (each dispatch costs ~600ns of serial sequencer time, so
            # fewer+bigger wins; a single dma_start fans over all 16 SDMA
            # engines anyway) ----
            bv_sb = singles.tile([128, COLS], F32)
            nc.sync.dma_start(out=bv_sb, in_=bv[:, :])
            bq_sb = singles.tile([128, 2], F32)
            nc.sync.dma_start(out=bq_sb, in_=bq[:, :])
            bk_sb = singles.tile([128, 2], F32)
            nc.sync.dma_start(out=bk_sb, in_=bk[:, :])
            if apply_mask:
                mm_sb = singles.tile([128, MC], F32)
                nc.sync.dma_start(out=mm_sb, in_=mm_in[:, :])

            wq_sb = singles.tile([128, KC, COLS], BF16)
            wk_sb = singles.tile([128, KC, COLS], BF16)
            wv_sb = singles.tile([128, KC, COLS], BF16)
            x_sb = singles.tile([128, KC, S], BF16)

            def dma_w(w_sb, w_ext):
                nc.sync.dma_start(
                    out=w_sb,
                    in_=w_ext.rearrange("(kc p) c -> p kc c", p=128))

            def dma_x(cq):
                csl = slice(cq * 512, (cq + 1) * 512)
                nc.sync.dma_start(
                    out=x_sb[:, :, csl],
                    in_=xT.rearrange("(kc p) c -> p kc c", p=128)[:, :, csl])

            dma_w(wk_sb, wk)
            dma_x(0)
            dma_w(wq_sb, wq)
            dma_w(wv_sb, wv)
            dma_x(1)
            dma_x(2)
            dma_x(3)

            # HAM warm-up: burn ~3us of dummy matmuls on the first-arriving
            # tensor (bv, tiny) so the PE is at full clock when real work
            # starts; output is never read.  (f32 matmuls run at half rate,
            # hence only 12.)
            warm_ps = psum.tile([128, 128], F32, tag="proj", name="warm_ps")
            for i in range(12):
                nc.tensor.matmul(warm_ps, lhsT=bv_sb[:, 0:128],
                                 rhs=bv_sb[:, 0:128],
                                 start=(i == 0), stop=(i == 11))

            # v_aug: [128, key_chunk, head, 128]; cols 64:128 are ones columns,
            # so the ctx matmul emits the softmax denominator replicated into
            # psum partitions 64:128 at no extra cost (matmul cost is N-bound)
            v_aug = singles.tile([128, MC, HPC, 128], BF16)
            nc.vector.memset(v_aug[:, :, :, 64:128], 1.0)

            kT = singles.tile([128, 2, S], BF16)
            qT = singles.tile([128, 2, S], BF16)

            def project_kqT(dst, w_sb, b_sb, p, c):
                """kT/qT projection for head pair p, 512-position chunk c.
                Returns two half-tasks (4 matmuls each) for fine-grained
                interleaving between attention patches."""
                csl = slice(c * 512, (c + 1) * 512)
                ps_box = []

                def half0():
                    ps = psum.tile([128, 512], F32, tag="proj",
                                   name=f"pt{nc.next_id()}")
                    ps_box.append(ps)
                    for kc in range(4):
                        nc.tensor.matmul(
                            ps, lhsT=w_sb[:, kc, p * 128:(p + 1) * 128],
                            rhs=x_sb[:, kc, csl], start=(kc == 0), stop=False)

                def half1():
                    ps = ps_box[0]
                    for kc in range(4, KC):
                        nc.tensor.matmul(
                            ps, lhsT=w_sb[:, kc, p * 128:(p + 1) * 128],
                            rhs=x_sb[:, kc, csl], start=False, stop=(kc == KC - 1))
                    nc.vector.tensor_tensor(
                        out=dst[:, p, csl], in0=ps,
                        in1=b_sb[:, p:p + 1].to_broadcast([128, 512]),
                        op=mybir.AluOpType.add)

                return half0, half1

            def project_v(mc):
                ps = psum.tile([128, COLS], F32, tag="proj",
                               name=f"pv{nc.next_id()}")
                for kc in range(KC):
                    nc.tensor.matmul(
                        ps, lhsT=x_sb[:, kc, mc * 128:(mc + 1) * 128],
                        rhs=wv_sb[:, kc, :], start=(kc == 0), stop=(kc == KC - 1))
                nc.vector.tensor_tensor(
                    out=v_aug[:, mc, :, 0:64],
                    in0=ps[:, :].rearrange("p (h d) -> p h d", h=HPC),
                    in1=bv_sb.rearrange("p (h d) -> p h d", h=HPC),
                    op=mybir.AluOpType.add)
                if apply_mask:
                    nc.vector.tensor_tensor(
                        out=v_aug[:, mc, :, :],
                        in0=v_aug[:, mc, :, :],
                        in1=mm_sb[:, mc:mc + 1, None].to_broadcast([128, HPC, 128]),
                        op=mybir.AluOpType.mult)

            # deferred-e store for the ramp (pair-0 qc1 exps run during qc0's
            # projection-heavy window; their ctx matmuls run later in qc2)
            e_defer = singles.tile([128, MC, 2, 512], BF16)

            def fill_and_exp(p, qc, kc, e_dst):
                """scoresT for key chunk kc via a row-tiled concurrent matmul
                pair, then one N=1024 Exp covering both heads."""
                qsl = slice(qc * 512, (qc + 1) * 512)
                ksl = slice(kc * 128, (kc + 1) * 128)
                s = psum.tile([128, 2, 512], F32, tag="sps",
                              name=f"s{nc.next_id()}")
                nc.tensor.matmul(s[:, 0, :], lhsT=kT[0:64, p, ksl],
                                 rhs=qT[0:64, p, qsl], start=True, stop=True)
                nc.tensor.matmul(s[:, 1, :], lhsT=kT[64:128, p, ksl],
                                 rhs=qT[64:128, p, qsl], start=True, stop=True)
                nc.scalar.activation(e_dst, s, mybir.ActivationFunctionType.Exp,
                                     scale=0.125)

            def ctx_mms(p, kc, e_ap, ctx_a, ctx_b, start, stop):
                ha, hb = 2 * p, 2 * p + 1
                nc.tensor.matmul(ctx_a, lhsT=v_aug[:, kc, ha, :],
                                 rhs=e_ap[0], start=start, stop=stop)
                nc.tensor.matmul(ctx_b, lhsT=v_aug[:, kc, hb, :],
                                 rhs=e_ap[1], start=start, stop=stop)

            def mk_ctx(p, qc):
                return (psum.tile([128, 512], F32, tag="ctx",
                                  name=f"ca{p}{qc}{nc.next_id()}"),
                        psum.tile([128, 512], F32, tag="ctx",
                                  name=f"cb{p}{qc}{nc.next_id()}"))

            def normalize(p, qc, ctx_pair):
                ha = 2 * p
                qsl = slice(qc * 512, (qc + 1) * 512)
                for h, ctx in ((ha, ctx_pair[0]), (ha + 1, ctx_pair[1])):
                    # one copy releases the ctx psum bank immediately;
                    # custom-DVE recip needs a base-0 input tile
                    g_sb = work.tile([128, 512], F32, tag="gctx",
                                     name=f"g{nc.next_id()}")
                    nc.vector.tensor_copy(out=g_sb, in_=ctx)
                    d0_sb = work.tile([64, 512], F32, tag="den0",
                                      name=f"d0{nc.next_id()}")
                    nc.vector.tensor_copy(out=d0_sb, in_=g_sb[64:128, :])
                    d_sb = work.tile([64, 512], F32, tag="den",
                                     name=f"d{nc.next_id()}")
                    nc.vector.reciprocal_approx_fast(out=d_sb, in_=d0_sb)
                    o_sb = work.tile([64, 512], F32, tag="outt",
                                     name=f"o{nc.next_id()}")
                    nc.vector.tensor_tensor(out=o_sb, in0=g_sb[0:64, :],
                                            in1=d_sb, op=mybir.AluOpType.mult)
                    nc.sync.dma_start(out=out_ext[h][:, qsl], in_=o_sb)

            def std_patch(p, qc, kc, ctx_pair, tasks):
                """fill + exp + ctx, with proj tasks emitted between the
                score fill and the ctx matmuls (so they hide under the exp)."""
                e = work.tile([128, 2, 512], BF16, tag="expT",
                              name=f"e{nc.next_id()}")
                fill_and_exp(p, qc, kc, e)
                for t in tasks:
                    t()
                ctx_mms(p, kc, (e[:, 0, :], e[:, 1, :]), *ctx_pair,
                        start=(kc == 0), stop=(kc == MC - 1))

            # ---- pair 0 ----
            # ramp: interleave qc0 (full patches + JIT v projections) with
            # qc1 (exp only, e parked in SBUF) so the scalar engine is fed
            # while the PE grinds through v/kT/qT projections.
            kqT_halves = {}
            for args in ((kT, wk_sb, bk_sb, 0, 1), (kT, wk_sb, bk_sb, 0, 2),
                         (kT, wk_sb, bk_sb, 0, 3), (qT, wq_sb, bq_sb, 0, 1),
                         (kT, wk_sb, bk_sb, 1, 0), (kT, wk_sb, bk_sb, 1, 1),
                         (kT, wk_sb, bk_sb, 1, 2), (kT, wk_sb, bk_sb, 1, 3),
                         (qT, wq_sb, bq_sb, 0, 2), (qT, wq_sb, bq_sb, 0, 3),
                         (qT, wq_sb, bq_sb, 1, 0), (qT, wq_sb, bq_sb, 1, 1),
                         (qT, wq_sb, bq_sb, 1, 2), (qT, wq_sb, bq_sb, 1, 3)):
                key = ("kT" if args[0] is kT else "qT", args[3], args[4])
                kqT_halves[key] = project_kqT(*args)

            h0, h1 = project_kqT(kT, wk_sb, bk_sb, 0, 0)
            h0(); h1()
            h0, h1 = project_kqT(qT, wq_sb, bq_sb, 0, 0)
            h0(); h1()

            # kT chunk c must be ready before ramp step 4c; spread the halves
            # over qc1-patch slots (emitted after that patch's score fill)
            ramp_post = {  # qc1-patch kc -> proj half specs
                2: [("kT", 0, 1, 0)],
                3: [("kT", 0, 1, 1)],
                5: [("kT", 0, 2, 0)],
                6: [("kT", 0, 2, 1)],
                9: [("kT", 0, 3, 0)],
                10: [("kT", 0, 3, 1)],
                12: [("qT", 0, 2, 0)],
                13: [("qT", 0, 2, 1)],
                14: [("qT", 0, 3, 0)],
                15: [("qT", 0, 3, 1)],
            }

            def get_half(spec):
                t, p_, c_, h_ = spec
                return kqT_halves[(t, p_, c_)][h_]

            ctx00 = mk_ctx(0, 0)
            for kc in range(MC):
                # qc0 patch: v(kc) emitted between fill and ctx
                std_patch(0, 0, kc, ctx00,
                          [lambda kc=kc: project_v(kc)])
                if kc == 0:
                    # qc1's queries must exist before its first score fill
                    get_half(("qT", 0, 1, 0))()
                    get_half(("qT", 0, 1, 1))()
                # qc1 patch: exp only, park e
                fill_and_exp(0, 1, kc, e_defer[:, kc, :, :])
                for spec in ramp_post.get(kc, ()):
                    get_half(spec)()
            normalize(0, 0, ctx00)

            # qc2: own patches + qc1's deferred ctx matmuls (2 per patch)
            ctx1 = (psum.tile([128, 512], F32, tag="proj", name="dca"),
                    psum.tile([128, 512], F32, tag="proj", name="dcb"))
            ctx2 = mk_ctx(0, 2)
            for kc in range(MC):
                std_patch(0, 2, kc, ctx2,
                          [lambda kc=kc: ctx_mms(
                              0, kc, (e_defer[:, kc, 0, :], e_defer[:, kc, 1, :]),
                              *ctx1, start=(kc == 0), stop=(kc == MC - 1))])
            normalize(0, 1, ctx1)
            normalize(0, 2, ctx2)

            # qc3: own patches + pair-1 kT chunks 0-2 and qT(p1,qc0)
            qc3_pre = {1: [("kT", 1, 0, 0)], 3: [("kT", 1, 0, 1)],
                       5: [("kT", 1, 1, 0)], 7: [("kT", 1, 1, 1)],
                       9: [("kT", 1, 2, 0)], 11: [("kT", 1, 2, 1)],
                       13: [("qT", 1, 0, 0)], 15: [("qT", 1, 0, 1)]}
            ctx3 = mk_ctx(0, 3)
            for kc in range(MC):
                std_patch(0, 3, kc, ctx3,
                          [get_half(s) for s in qc3_pre.get(kc, ())])
            normalize(0, 3, ctx3)

            # ---- pair 1 ----
            p1_pre = {
                (0, 1): [("kT", 1, 3, 0)], (0, 3): [("kT", 1, 3, 1)],
                (0, 5): [("qT", 1, 1, 0)], (0, 7): [("qT", 1, 1, 1)],
                (1, 1): [("qT", 1, 2, 0)], (1, 3): [("qT", 1, 2, 1)],
                (2, 1): [("qT", 1, 3, 0)], (2, 3): [("qT", 1, 3, 1)],
            }
            for qc in range(QC):
                ctxp = mk_ctx(1, qc)
                for kc in range(MC):
                    std_patch(1, qc, kc, ctxp,
                              [get_half(s) for s in p1_pre.get((qc, kc), ())])
                normalize(1, qc, ctxp)

    nc.compile()
    return nc


def _get_nc(apply_mask: bool) -> bass.Bass:
    if apply_mask not in _CACHE:
        _CACHE[apply_mask] = build(apply_mask)
    return _CACHE[apply_mask]


def _in_maps(x, mask, Wq, bq, Wk, bk, Wv, bv, apply_mask):
    xT_b = [np.ascontiguousarray(x[b].T).astype(np_bf16) for b in range(B)]
    maps = []
    for c in range(NCORES):
        b, hg = c // 4, c % 4
        cs = slice(hg * COLS, (hg + 1) * COLS)
        m = {
            "xT": xT_b[b],
            "wq": np.ascontiguousarray(Wq[:, cs]).astype(np_bf16),
            "wk": np.ascontiguousarray(Wk[:, cs]).astype(np_bf16),
            "wv": np.ascontiguousarray(Wv[:, cs]).astype(np_bf16),
            "bq": np.ascontiguousarray(bq[cs].reshape(2, 128).T).astype(np.float32),
            "bk": np.ascontiguousarray(bk[cs].reshape(2, 128).T).astype(np.float32),
            "bv": np.ascontiguousarray(
                np.broadcast_to(bv[cs], (128, COLS))).astype(np.float32),
        }
        if apply_mask:
            m["maskm"] = np.ascontiguousarray(
                mask[b].astype(np.float32).reshape(MC, 128).T)
        maps.append(m)
    return maps


def _ensure_ntff_hook():
    """The agent image's antenv lacks axon_hooks; synthesize it so
    run_bass_kernel_spmd(trace=True) can reach the axon NTFF profiler."""
    import sys as _sys
    import types as _types
    try:
        from antenv import axon_hooks  # noqa: F401
        return
    except ImportError:
        pass
    import antenv
    mod = _types.ModuleType("antenv.axon_hooks")
    _hook = [None]
    mod.set_axon_ntff_profile_hook = lambda h: _hook.__setitem__(0, h)
    mod.get_axon_ntff_profile_hook = lambda: _hook[0]
    _sys.modules["antenv.axon_hooks"] = mod
    antenv.axon_hooks = mod
    from trn_agent_boot.trn_boot import _ntff_profile_via_ctypes
    mod.set_axon_ntff_profile_hook(
        _ntff_profile_via_ctypes("/opt/axon/libaxon_pjrt.so"))


def run(inputs: dict, trace: bool = False):
    if trace:
        _ensure_ntff_hook()
    x = np.asarray(inputs["x"], dtype=np.float32)
    mask = np.asarray(inputs["mask"])
    apply_mask = not bool((mask == 1).all())
    nc = _get_nc(apply_mask)
    maps = _in_maps(x, mask, np.asarray(inputs["Wq"], np.float32),
                    np.asarray(inputs["bq"], np.float32),
                    np.asarray(inputs["Wk"], np.float32),
                    np.asarray(inputs["bk"], np.float32),
                    np.asarray(inputs["Wv"], np.float32),
                    np.asarray(inputs["bv"], np.float32), apply_mask)
    res = run_bass_kernel_spmd(nc, maps, core_ids=list(range(NCORES)), trace=trace)
    out = np.empty((B, S, HID), dtype=np.float32)
    for c in range(NCORES):
        b, hg = c // 4, c % 4
        cs = slice(hg * COLS, (hg + 1) * COLS)
        ctxT = res.results[c]["out"]          # [HPC, D, S]
        out[b, :, cs] = ctxT.transpose(2, 0, 1).reshape(S, COLS)
    return out, res


def kernel(**inputs) -> np.ndarray:
    out, _ = run(inputs)
    return out
